# revision 8
# baseline (speedup 1.0000x reference)
"""Trainium2 Bass kernel for nn_CTMBridge (scatter_memory).

Math: the (B,T,S,WD) cumsum "slot memory" factors into chunked linear
attention.  Per batch:
    h    = rms_norm(x)
    nov  = mean((h-x0)^2, -1);  sal = mean(h*h, -1)  (both normalized by
           running means over time -> gate signal g)
    tg   = sigmoid(h @ Wg.T + g)
    sl   = h @ Wsel.T;  ww = softmax(sl + eg*g)*tg;  rw = softmax(sl)
    wv   = h @ Wws.T
    den_ex[t,j] = sum_{t'<t} ww[t',j];   a = (rw@mix)/max(den_ex,1e-6)
    ctx[t,:] = sum_{t'<t} (a[t,:]. ww[t',:]) wv[t',:]      (linear attention)
    out  = x + ctm * tg * (ctx @ Wm.T)
Chunked over T with C=128: intra-chunk via strict-causal (C,C) matmuls,
inter-chunk via running state M (S,WD) and den (S,).

Sharding: 8 cores.  v1 (USE_CC=False): core k computes batch k//4 fully
(replicated); host picks cores 0 and 4.  v2 (USE_CC=True): each batch's
time axis is split over 4 cores; two tiny AllGathers exchange (a) the
novelty/salience running-sum prefixes and (b) the (S,WD) slot-state
prefixes between the time shards.
"""
import json
import numpy as np
from contextlib import ExitStack

import concourse.bass as bass
import concourse.mybir as mybir
import concourse.tile as tile
import concourse.bass_utils as bass_utils
import concourse.bass2jax as bass2jax
from concourse.bass_utils import run_bass_kernel_spmd
from concourse.masks import make_identity, make_upper_triangular


# --- workaround: this walrus build only accepts a single sync-wait per
# instruction on several instruction formats.  Tile attaches every needed
# cross-engine wait to the consuming instruction, so split the excess into
# standalone EventSemaphore instructions (engines are in-order, so waiting
# immediately before the instruction is equivalent).
_WAIT_LIMIT = 1


def _split_excess_waits(bir_bytes: bytes) -> bytes:
    bir = json.loads(bir_bytes)
    n = 0
    for f in bir.get("functions", []):
        for blk in f.get("blocks", []):
            out = []
            for ins in blk.get("instructions", []):
                si = ins.get("sync_info")
                waits = (si or {}).get("on_wait") or []
                eng = ins.get("engine")
                if len(waits) > _WAIT_LIMIT and eng and eng != "Unassigned":
                    for w in waits[:-_WAIT_LIMIT]:
                        n += 1
                        ev = {
                            "engine": eng,
                            "ins": [],
                            "outs": [],
                            "name": f"wsplit{n}_{ins['name']}",
                            "opcode": "EventSemaphore",
                            "sync_info": {"on_update": [], "on_wait": [w]},
                        }
                        if "debug" in ins:
                            ev["debug"] = ins["debug"]
                        out.append(ev)
                    si["on_wait"] = waits[-_WAIT_LIMIT:]
                out.append(ins)
            blk["instructions"] = out
    return json.dumps(bir).encode()


_orig_compile_bir_kernel = bass_utils.compile_bir_kernel


def _patched_compile_bir_kernel(bir_json, tmpdir, neff_name="file.neff"):
    if isinstance(bir_json, (bytes, bytearray)):
        bir_json = _split_excess_waits(bytes(bir_json))
    elif isinstance(bir_json, str):
        bir_json = _split_excess_waits(bir_json.encode()).decode()
    return _orig_compile_bir_kernel(bir_json, tmpdir, neff_name)


bass_utils.compile_bir_kernel = _patched_compile_bir_kernel
bass2jax.compile_bir_kernel = _patched_compile_bir_kernel

B, T, D, S, WD = 2, 2048, 1024, 64, 256
C = 128                      # tokens per chunk (= partition count)
DT = D // C                  # 8 d-tiles
WT = WD // C                 # 2 wd-tiles
NPROJ = S + 1 + WD           # fused projection width: [sl | ghdot | wv]
NCORES = 8
CORES_PER_B = NCORES // B
F32 = mybir.dt.float32
EPS = float(np.finfo(np.float32).eps)
AF = mybir.ActivationFunctionType
ALU = mybir.AluOpType
AX = mybir.AxisListType

USE_CC = False               # v2: time-sharded with AllGather prefix exchange


def _build(t_loc: int, use_cc: bool) -> bass.Bass:
    NCH = t_loc // C
    nc = bass.Bass()

    x_in = nc.dram_tensor("x", [t_loc, D], F32, kind="ExternalInput")
    x0_in = nc.dram_tensor("x0", [t_loc, D], F32, kind="ExternalInput")
    wproj_in = nc.dram_tensor("wproj", [D, NPROJ], F32, kind="ExternalInput")
    wm_in = nc.dram_tensor("wm", [WD, D], F32, kind="ExternalInput")
    mix_in = nc.dram_tensor("mix", [S, S], F32, kind="ExternalInput")
    eg_in = nc.dram_tensor("eg", [1, S], F32, kind="ExternalInput")
    out_dram = nc.dram_tensor("out", [t_loc, D], F32, kind="ExternalOutput")
    if use_cc:
        # pmask[k] = 1.0 for cores of the same batch whose time-slice precedes ours
        pmask_in = nc.dram_tensor("pmask", [NCORES, 1], F32, kind="ExternalInput")

    with tile.TileContext(nc) as tc, ExitStack() as ctx:
        if use_cc:
            dramp = ctx.enter_context(tc.tile_pool(name="dramp", bufs=1,
                                                   space="DRAM"))
            ag1_in = dramp.tile([1, 2], F32)
            ag1_out = dramp.tile([NCORES, 2], F32, addr_space="Shared")
            ag2_in = dramp.tile([S + 1, WD], F32)
            ag2_out = dramp.tile([NCORES, S + 1, WD], F32, addr_space="Shared")
        sing = ctx.enter_context(tc.tile_pool(name="sing", bufs=1))
        work = ctx.enter_context(tc.tile_pool(name="work", bufs=2))
        w3 = ctx.enter_context(tc.tile_pool(name="w3", bufs=2))
        st = ctx.enter_context(tc.tile_pool(name="st", bufs=3))
        ps_tp = ctx.enter_context(tc.tile_pool(name="ps_tp", bufs=2, space="PSUM"))
        ps_md = ctx.enter_context(tc.tile_pool(name="ps_md", bufs=2, space="PSUM"))
        ps_o = ctx.enter_context(tc.tile_pool(name="ps_o", bufs=2, space="PSUM"))

        # ---- constants ----
        ident = sing.tile([C, C], F32)
        make_identity(nc, ident)
        ustrict = sing.tile([C, C], F32)          # u[t',t] = 1 iff t' < t
        make_upper_triangular(nc, ustrict, val=1.0, diag=False)
        uincl = sing.tile([C, C], F32)            # u[t',t] = 1 iff t' <= t
        make_upper_triangular(nc, uincl, val=1.0, diag=True)
        ones_col = sing.tile([C, 1], F32)
        nc.vector.memset(ones_col, 1.0)
        ones_row = sing.tile([1, C], F32)
        nc.vector.memset(ones_row, 1.0)
        zeros_row = sing.tile([1, NCH], F32)
        nc.vector.memset(zeros_row, 0.0)

        # steps (t+1) and their reciprocals, replicated for the 2 kinds
        steps_i = sing.tile([C, NCH], mybir.dt.int32)
        base0 = 1  # v2: per-core global offset is added via an input instead
        toff = sing.tile([C, 1], F32)  # global token offset of this core (input)
        rsteps2 = sing.tile([C, 2, NCH], F32)
        if use_cc:
            toff_in = nc.dram_tensor("toff", [1, 1], F32, kind="ExternalInput")
            nc.gpsimd.dma_start(out=toff, in_=toff_in[0:1, :].to_broadcast([C, 1]))
        else:
            nc.vector.memset(toff, 0.0)
        nc.gpsimd.iota(steps_i, pattern=[[C, NCH]], base=base0, channel_multiplier=1)
        stepsf = sing.tile([C, NCH], F32)
        nc.vector.tensor_copy(stepsf, steps_i)               # int -> float
        nc.vector.tensor_scalar_add(stepsf, stepsf, toff[:, 0:1])  # + global offset
        nc.vector.reciprocal(stepsf, stepsf)
        nc.vector.tensor_copy(rsteps2[:, 0, :], stepsf)
        nc.vector.tensor_copy(rsteps2[:, 1, :], stepsf)

        # ---- weights ----
        wproj_sb = sing.tile([C, DT, NPROJ], F32)
        nc.sync.dma_start(out=wproj_sb,
                          in_=wproj_in.rearrange("(a p) n -> p a n", p=C))
        wm_sb = sing.tile([C, WT, D], F32)
        nc.sync.dma_start(out=wm_sb, in_=wm_in.rearrange("(a p) n -> p a n", p=C))
        mix_sb = sing.tile([S, S], F32)
        nc.sync.dma_start(out=mix_sb, in_=mix_in[:, :])
        egb = sing.tile([C, S], F32)
        nc.gpsimd.dma_start(out=egb, in_=eg_in[0:1, :].to_broadcast([C, S]))
        if use_cc:
            pmask_col = sing.tile([NCORES, 1], F32)
            nc.sync.dma_start(out=pmask_col, in_=pmask_in[:, :])

        # ---- persistent per-core arrays ----
        x_all = sing.tile([C, NCH, D], F32)
        sl_all = sing.tile([C, NCH, S], F32)
        wv_all = sing.tile([C, NCH, WD], F32)
        ghdot_all = sing.tile([C, NCH], F32)
        novsal = sing.tile([C, 2, NCH], F32)
        g_all = sing.tile([C, NCH], F32)
        tg_all = sing.tile([C, NCH], F32)
        ww_all = sing.tile([C, NCH, S], F32)
        carr_i = sing.tile([1, 2, NCH], F32)
        carries = sing.tile([1, 2, NCH], F32)
        m_sb = sing.tile([S, WD], F32)
        den_sb = sing.tile([1, S], F32)

        # ================= phase 1: token-local heavy pipeline =================
        with nc.named_scope("phase1"):
            for c in range(NCH):
                xs = x_all[:, c, :]
                nc.sync.dma_start(out=xs, in_=x_in[c * C:(c + 1) * C, :])
                x0t = work.tile([C, D], F32, tag="x0")
                nc.sync.dma_start(out=x0t, in_=x0_in[c * C:(c + 1) * C, :])

                ht = work.tile([C, D], F32, tag="h")
                xsq = st.tile([C, 1], F32, tag="xsq")
                # sum(x^2) along D (ht is scratch for the squares)
                nc.scalar.activation(ht, xs, AF.Square, accum_out=xsq)
                v1t = st.tile([C, 1], F32, tag="v1")
                nc.vector.tensor_scalar(v1t, xsq, 1.0 / D, EPS,
                                        op0=ALU.mult, op1=ALU.add)
                rv = st.tile([C, 1], F32, tag="rv")
                nc.vector.reciprocal(rv, v1t)
                # salience_raw = (sum(x^2)/D) * 1/(meansq+eps)
                nc.vector.scalar_tensor_tensor(novsal[:, 1, c:c + 1], xsq, 1.0 / D,
                                               rv, op0=ALU.mult, op1=ALU.mult)
                rr = st.tile([C, 1], F32, tag="rr")
                nc.scalar.sqrt(rr, rv)                       # rsqrt(meansq+eps)
                nc.vector.tensor_scalar_mul(ht, xs, rr)      # h = x * r
                nc.gpsimd.tensor_sub(x0t, ht, x0t)           # d = h - x0
                nsum = st.tile([C, 1], F32, tag="ns")
                nc.scalar.activation(x0t, x0t, AF.Square, accum_out=nsum)
                nc.vector.tensor_scalar_mul(novsal[:, 0, c:c + 1], nsum, 1.0 / D)

                hT = work.tile([C, DT, C], F32, tag="hT")
                for dt in range(DT):
                    pt = ps_tp.tile([C, C], F32, tag="tp")
                    nc.tensor.transpose(pt, ht[:, dt * C:(dt + 1) * C], ident)
                    if dt % 2 == 0:
                        nc.vector.tensor_copy(hT[:, dt, :], pt)
                    else:
                        nc.scalar.copy(hT[:, dt, :], pt)

                pproj = ps_md.tile([C, NPROJ], F32, tag="md")
                for dt in range(DT):
                    nc.tensor.matmul(pproj, hT[:, dt, :], wproj_sb[:, dt, :],
                                     start=(dt == 0), stop=(dt == DT - 1))
                nc.scalar.copy(sl_all[:, c, :], pproj[:, 0:S])
                nc.scalar.copy(ghdot_all[:, c:c + 1], pproj[:, S:S + 1])
                nc.vector.tensor_copy(wv_all[:, c, :], pproj[:, S + 1:NPROJ])

        # ============ phase 2: running-mean chain + gates (one shot) ============
        with nc.named_scope("phase2"):
            ptot = ps_md.tile([1, 2, NCH], F32, tag="md")
            nc.tensor.matmul(ptot, ones_col, novsal, start=True, stop=True)
            if use_cc:
                # my totals = sum over chunks; AllGather; masked prefix sum
                mytot = sing.tile([1, 2], F32)
                nc.vector.tensor_reduce(mytot, ptot, axis=AX.X, op=ALU.add)
                nc.sync.dma_start(out=ag1_in[:, :], in_=mytot)
                nc.gpsimd.collective_compute(
                    "AllGather", ALU.bypass, ins=[ag1_in[:]], outs=[ag1_out[:]],
                    replica_groups=[list(range(NCORES))])
                tots8 = sing.tile([NCORES, 2], F32)
                nc.sync.dma_start(out=tots8, in_=ag1_out[:, :])
                ppref = ps_md.tile([1, 2], F32, tag="md")
                nc.tensor.matmul(ppref, pmask_col, tots8, start=True, stop=True)
                pref = sing.tile([1, 2], F32)
                nc.vector.tensor_copy(pref, ppref)
            for k in range(2):
                nc.vector.tensor_tensor_scan(carr_i[:, k, :], ptot[:, k, :],
                                             zeros_row, 0.0,
                                             op0=ALU.add, op1=ALU.add)
            nc.vector.memset(carries[:, :, 0:1], 0.0)
            for k in range(2):
                nc.vector.tensor_copy(carries[:, k, 1:NCH], carr_i[:, k, 0:NCH - 1])
            if use_cc:
                for k in range(2):
                    nc.vector.tensor_scalar_add(carries[:, k, :], carries[:, k, :],
                                                pref[:, k:k + 1])
            pcum = ps_md.tile([C, 2, NCH], F32, tag="md")
            nc.tensor.matmul(pcum, uincl, novsal, start=True, stop=False)
            nc.tensor.matmul(pcum, ones_row, carries, start=False, stop=True)
            rm = sing.tile([C, 2, NCH], F32)
            nc.vector.tensor_mul(rm, pcum, rsteps2)          # cum/(t+1)
            nc.vector.tensor_scalar_max(rm, rm, 1e-6)
            nc.vector.reciprocal(rm, rm)
            nc.vector.tensor_mul(rm, rm, novsal)             # normalized ratios
            nc.vector.tensor_add(g_all, rm[:, 0, :], rm[:, 1, :])
            g2t = sing.tile([C, NCH], F32)
            nc.vector.tensor_add(g2t, g_all, ghdot_all)
            nc.scalar.activation(tg_all, g2t, AF.Sigmoid)

        # ======= phase 2.5: write weights ww for all chunks (+ state deltas) =======
        with nc.named_scope("phase2p5"):
            for c in range(NCH):
                g_col = g_all[:, c:c + 1]
                tg_col = tg_all[:, c:c + 1]
                wl = w3.tile([C, S], F32, tag="wl")
                nc.vector.scalar_tensor_tensor(wl, egb, g_col, sl_all[:, c, :],
                                               op0=ALU.mult, op1=ALU.add)
                mx = st.tile([C, 1], F32, tag="mx")
                nc.vector.tensor_reduce(mx, wl, axis=AX.X, op=ALU.max, negate=True)
                eww = w3.tile([C, S], F32, tag="eww")
                sww = st.tile([C, 1], F32, tag="sww")
                nc.scalar.activation(eww, wl, AF.Exp, bias=mx, accum_out=sww)
                rs = st.tile([C, 1], F32, tag="rs")
                nc.vector.reciprocal(rs, sww)
                sw = st.tile([C, 1], F32, tag="sw")
                nc.vector.tensor_mul(sw, rs, tg_col)
                nc.vector.tensor_scalar_mul(ww_all[:, c, :], eww, sw)
            if use_cc:
                pmd_d = ps_md.tile([S, WD], F32, tag="md")
                pdd_d = ps_md.tile([1, S], F32, tag="md")
                for c in range(NCH):
                    nc.tensor.matmul(pmd_d, ww_all[:, c, :], wv_all[:, c, :],
                                     start=(c == 0), stop=(c == NCH - 1))
                for c in range(NCH):
                    nc.tensor.matmul(pdd_d, ones_col, ww_all[:, c, :],
                                     start=(c == 0), stop=(c == NCH - 1))
                mden_d = sing.tile([S + 1, WD], F32)
                nc.vector.tensor_copy(mden_d[0:S, :], pmd_d)
                nc.vector.memset(mden_d[S:S + 1, :], 0.0)
                nc.vector.tensor_copy(mden_d[S:S + 1, 0:S], pdd_d)
                nc.sync.dma_start(out=ag2_in[:, :], in_=mden_d)
                nc.gpsimd.collective_compute(
                    "AllGather", ALU.bypass, ins=[ag2_in[:]], outs=[ag2_out[:]],
                    replica_groups=[list(range(NCORES))])
                # gathered (k, p, d) -> sbuf [p, k, d]
                g2 = sing.tile([S + 1, NCORES, WD], F32)
                nc.sync.dma_start(out=g2,
                                  in_=ag2_out.rearrange("k p n -> p k n"))
                pmask_b = sing.tile([S + 1, NCORES], F32)
                nc.gpsimd.dma_start(
                    out=pmask_b,
                    in_=pmask_in.rearrange("k one -> one k")[0:1, :]
                        .to_broadcast([S + 1, NCORES]))
                nc.vector.memset(m_sb, 0.0)
                nc.vector.memset(den_sb, 0.0)
                tmpm = sing.tile([S + 1, WD], F32)
                for k in range(NCORES):
                    nc.vector.tensor_scalar_mul(tmpm, g2[:, k, :],
                                                pmask_b[:, k:k + 1])
                    nc.vector.tensor_add(m_sb, m_sb, tmpm[0:S, :])
                    nc.vector.tensor_add(den_sb, den_sb, tmpm[S:S + 1, 0:S])
            else:
                nc.vector.memset(m_sb, 0.0)
                nc.vector.memset(den_sb, 0.0)

        # ================= phase 3: attention + output =================
        with nc.named_scope("phase3"):
            for c in range(NCH):
                tg_col = tg_all[:, c:c + 1]
                ww = ww_all[:, c, :]
                # read weights softmax (no gate signal)
                mx2 = st.tile([C, 1], F32, tag="mx2")
                nc.vector.tensor_reduce(mx2, sl_all[:, c, :], axis=AX.X,
                                        op=ALU.max, negate=True)
                erw = w3.tile([C, S], F32, tag="erw")
                srw = st.tile([C, 1], F32, tag="srw")
                nc.scalar.activation(erw, sl_all[:, c, :], AF.Exp, bias=mx2,
                                     accum_out=srw)
                rs2 = st.tile([C, 1], F32, tag="rs2")
                nc.vector.reciprocal(rs2, srw)
                rw = w3.tile([C, S], F32, tag="rw")
                nc.vector.tensor_scalar_mul(rw, erw, rs2)
                # rw2 = rw @ mix  (token-major)
                prwT = ps_tp.tile([S, C], F32, tag="tp")
                nc.tensor.transpose(prwT, rw, ident)
                rwT = w3.tile([S, C], F32, tag="rwT")
                nc.vector.tensor_copy(rwT, prwT)
                prw2 = ps_md.tile([C, S], F32, tag="md")
                nc.tensor.matmul(prw2, rwT, mix_sb, start=True, stop=True)
                # den_ex = strict-cumsum(ww) + den_acc
                pden = ps_md.tile([C, S], F32, tag="md")
                nc.tensor.matmul(pden, ustrict, ww, start=True, stop=False)
                nc.tensor.matmul(pden, ones_row, den_sb, start=False, stop=True)
                dcl = w3.tile([C, S], F32, tag="dcl")
                nc.vector.tensor_scalar_max(dcl, pden, 1e-6)
                nc.vector.reciprocal(dcl, dcl)
                a_t = w3.tile([C, S], F32, tag="a")
                nc.vector.tensor_mul(a_t, prw2, dcl)
                # transposes for the attention matmuls
                paT = ps_tp.tile([S, C], F32, tag="tp")
                nc.tensor.transpose(paT, a_t, ident)
                aT = w3.tile([S, C], F32, tag="aT")
                nc.vector.tensor_copy(aT, paT)
                pwwT = ps_tp.tile([S, C], F32, tag="tp")
                nc.tensor.transpose(pwwT, ww, ident)
                wwT = w3.tile([S, C], F32, tag="wwT")
                nc.scalar.copy(wwT, pwwT)
                # K^T (t',t) then strict-causal mask
                pKT = ps_md.tile([C, C], F32, tag="md")
                nc.tensor.matmul(pKT, wwT, aT, start=True, stop=True)
                KTm = w3.tile([C, C], F32, tag="KTm")
                nc.vector.tensor_mul(KTm, pKT, ustrict)
                # ctx = K^T.T @ wv + a @ M
                pctx = ps_md.tile([C, WD], F32, tag="md")
                nc.tensor.matmul(pctx, KTm, wv_all[:, c, :], start=True, stop=False)
                nc.tensor.matmul(pctx, aT, m_sb, start=False, stop=True)
                ctxs = w3.tile([C, WD], F32, tag="ctx")
                nc.vector.tensor_copy(ctxs, pctx)
                # state update (after pden/pctx consumed den_sb/m_sb)
                pMd = ps_md.tile([S, WD], F32, tag="md")
                nc.tensor.matmul(pMd, ww, wv_all[:, c, :], start=True, stop=True)
                nc.vector.tensor_add(m_sb, m_sb, pMd)
                pdd = ps_md.tile([1, S], F32, tag="md")
                nc.tensor.matmul(pdd, ones_col, ww, start=True, stop=True)
                nc.vector.tensor_add(den_sb, den_sb, pdd)
                # o = ctx @ Wm'^T ;  out = x + tg*o
                ctxT = w3.tile([C, WT, C], F32, tag="ctxT")
                for wt in range(WT):
                    pcT = ps_tp.tile([C, C], F32, tag="tp")
                    nc.tensor.transpose(pcT, ctxs[:, wt * C:(wt + 1) * C], ident)
                    nc.scalar.copy(ctxT[:, wt, :], pcT)
                po = ps_o.tile([C, D], F32, tag="o")
                for nb in range(2):
                    for wt in range(WT):
                        nc.tensor.matmul(po[:, nb * 512:(nb + 1) * 512],
                                         ctxT[:, wt, :],
                                         wm_sb[:, wt, nb * 512:(nb + 1) * 512],
                                         start=(wt == 0), stop=(wt == WT - 1))
                outs = w3.tile([C, D], F32, tag="outs")
                nc.vector.scalar_tensor_tensor(outs, po, tg_col, x_all[:, c, :],
                                               op0=ALU.mult, op1=ALU.add)
                nc.sync.dma_start(out=out_dram[c * C:(c + 1) * C, :], in_=outs)

    return nc


_cache: dict = {}


def _get_nc(t_loc: int, use_cc: bool):
    key = (t_loc, use_cc)
    if key not in _cache:
        _cache[key] = _build(t_loc, use_cc)
    return _cache[key]


def kernel_with_results(x, x0, W_select, W_workspace, W_model, W_gate, slot_mix,
                        error_gain, ctm_scale, trace=False, **rkw):
    x = np.ascontiguousarray(np.asarray(x, np.float32))
    x0 = np.ascontiguousarray(np.asarray(x0, np.float32))
    wproj = np.ascontiguousarray(
        np.concatenate([np.asarray(W_select, np.float32),
                        np.asarray(W_gate, np.float32),
                        np.asarray(W_workspace, np.float32)], axis=0).T)
    wm = np.ascontiguousarray(
        (np.asarray(ctm_scale, np.float32)[:, None]
         * np.asarray(W_model, np.float32)).T)
    mix = np.ascontiguousarray(np.asarray(slot_mix, np.float32))
    eg = np.ascontiguousarray(np.asarray(error_gain, np.float32).reshape(1, S))

    t_loc = T // CORES_PER_B if USE_CC else T
    nc = _get_nc(t_loc, USE_CC)
    in_maps = []
    for k in range(NCORES):
        b, q = divmod(k, CORES_PER_B)
        sl_t = slice(q * t_loc, (q + 1) * t_loc) if USE_CC else slice(0, T)
        m = {"x": np.ascontiguousarray(x[b, sl_t]),
             "x0": np.ascontiguousarray(x0[b, sl_t]),
             "wproj": wproj, "wm": wm, "mix": mix, "eg": eg}
        if USE_CC:
            pm = np.zeros((NCORES, 1), np.float32)
            pm[b * CORES_PER_B:k, 0] = 1.0
            m["pmask"] = pm
            m["toff"] = np.full((1, 1), q * t_loc, np.float32)
        in_maps.append(m)
    res = run_bass_kernel_spmd(nc, in_maps, core_ids=list(range(NCORES)),
                               trace=trace, **rkw)
    outs = [r["out"] for r in res.results]
    if USE_CC:
        out = np.stack([np.concatenate(outs[b * CORES_PER_B:(b + 1) * CORES_PER_B],
                                       axis=0) for b in range(B)])
    else:
        out = np.stack([outs[0], outs[CORES_PER_B]])
    return out, res


def kernel(**inputs) -> np.ndarray:
    out, _ = kernel_with_results(**inputs)
    return out


# revision 10
# speedup vs baseline: 1.8414x; 1.8414x over previous
"""Trainium2 Bass kernel for nn_CTMBridge (scatter_memory).

Math: the (B,T,S,WD) cumsum "slot memory" factors into chunked linear
attention.  Per batch:
    h    = rms_norm(x)
    nov  = mean((h-x0)^2, -1);  sal = mean(h*h, -1)  (both normalized by
           running means over time -> gate signal g)
    tg   = sigmoid(h @ Wg.T + g)
    sl   = h @ Wsel.T;  ww = softmax(sl + eg*g)*tg;  rw = softmax(sl)
    wv   = h @ Wws.T
    den_ex[t,j] = sum_{t'<t} ww[t',j];   a = (rw@mix)/max(den_ex,1e-6)
    ctx[t,:] = sum_{t'<t} (a[t,:]. ww[t',:]) wv[t',:]      (linear attention)
    out  = x + ctm * tg * (ctx @ Wm.T)
Chunked over T with C=128: intra-chunk via strict-causal (C,C) matmuls,
inter-chunk via running state M (S,WD) and den (S,).

Sharding: 8 cores.  v1 (USE_CC=False): core k computes batch k//4 fully
(replicated); host picks cores 0 and 4.  v2 (USE_CC=True): each batch's
time axis is split over 4 cores; two tiny AllGathers exchange (a) the
novelty/salience running-sum prefixes and (b) the (S,WD) slot-state
prefixes between the time shards.
"""
import json
import numpy as np
from contextlib import ExitStack

import concourse.bass as bass
import concourse.mybir as mybir
import concourse.tile as tile
import concourse.bass_utils as bass_utils
import concourse.bass2jax as bass2jax
from concourse.bass_utils import run_bass_kernel_spmd
from concourse.masks import make_identity, make_upper_triangular


# --- workaround: this walrus build only accepts a single sync-wait per
# instruction on several instruction formats.  Tile attaches every needed
# cross-engine wait to the consuming instruction, so split the excess into
# standalone EventSemaphore instructions (engines are in-order, so waiting
# immediately before the instruction is equivalent).
_WAIT_LIMIT = 1


def _split_excess_waits(bir_bytes: bytes) -> bytes:
    bir = json.loads(bir_bytes)
    n = 0
    for f in bir.get("functions", []):
        for blk in f.get("blocks", []):
            out = []
            for ins in blk.get("instructions", []):
                si = ins.get("sync_info")
                waits = (si or {}).get("on_wait") or []
                eng = ins.get("engine")
                if len(waits) > _WAIT_LIMIT and eng and eng != "Unassigned":
                    for w in waits[:-_WAIT_LIMIT]:
                        n += 1
                        ev = {
                            "engine": eng,
                            "ins": [],
                            "outs": [],
                            "name": f"wsplit{n}_{ins['name']}",
                            "opcode": "EventSemaphore",
                            "sync_info": {"on_update": [], "on_wait": [w]},
                        }
                        if "debug" in ins:
                            ev["debug"] = ins["debug"]
                        out.append(ev)
                    si["on_wait"] = waits[-_WAIT_LIMIT:]
                out.append(ins)
            blk["instructions"] = out
    return json.dumps(bir).encode()


_orig_compile_bir_kernel = bass_utils.compile_bir_kernel


def _patched_compile_bir_kernel(bir_json, tmpdir, neff_name="file.neff"):
    if isinstance(bir_json, (bytes, bytearray)):
        bir_json = _split_excess_waits(bytes(bir_json))
    elif isinstance(bir_json, str):
        bir_json = _split_excess_waits(bir_json.encode()).decode()
    return _orig_compile_bir_kernel(bir_json, tmpdir, neff_name)


bass_utils.compile_bir_kernel = _patched_compile_bir_kernel
bass2jax.compile_bir_kernel = _patched_compile_bir_kernel

B, T, D, S, WD = 2, 2048, 1024, 64, 256
C = 128                      # tokens per chunk (= partition count)
DT = D // C                  # 8 d-tiles
WT = WD // C                 # 2 wd-tiles
NPROJ = S + 1 + WD           # fused projection width: [sl | ghdot | wv]
NCORES = 8
CORES_PER_B = NCORES // B
F32 = mybir.dt.float32
EPS = float(np.finfo(np.float32).eps)
AF = mybir.ActivationFunctionType
ALU = mybir.AluOpType
AX = mybir.AxisListType

USE_CC = True                # v2: time-sharded with AllGather prefix exchange


def _build(t_loc: int, use_cc: bool) -> bass.Bass:
    NCH = t_loc // C
    nc = bass.Bass()

    x_in = nc.dram_tensor("x", [t_loc, D], F32, kind="ExternalInput")
    x0_in = nc.dram_tensor("x0", [t_loc, D], F32, kind="ExternalInput")
    wproj_in = nc.dram_tensor("wproj", [D, NPROJ], F32, kind="ExternalInput")
    wm_in = nc.dram_tensor("wm", [WD, D], F32, kind="ExternalInput")
    mix_in = nc.dram_tensor("mix", [S, S], F32, kind="ExternalInput")
    eg_in = nc.dram_tensor("eg", [1, S], F32, kind="ExternalInput")
    out_dram = nc.dram_tensor("out", [t_loc, D], F32, kind="ExternalOutput")
    if use_cc:
        # pmask[k] = 1.0 for cores of the same batch whose time-slice precedes ours
        pmask_in = nc.dram_tensor("pmask", [NCORES, 1], F32, kind="ExternalInput")

    with tile.TileContext(nc) as tc, ExitStack() as ctx:
        if use_cc:
            dramp = ctx.enter_context(tc.tile_pool(name="dramp", bufs=1,
                                                   space="DRAM"))
            ag1_in = dramp.tile([1, 2], F32)
            ag1_out = dramp.tile([NCORES, 2], F32, addr_space="Shared")
            ag2_in = dramp.tile([S + 1, WD], F32)
            ag2_out = dramp.tile([NCORES, S + 1, WD], F32, addr_space="Shared")
        sing = ctx.enter_context(tc.tile_pool(name="sing", bufs=1))
        work = ctx.enter_context(tc.tile_pool(name="work", bufs=2))
        w3 = ctx.enter_context(tc.tile_pool(name="w3", bufs=2))
        st = ctx.enter_context(tc.tile_pool(name="st", bufs=3))
        ps_tp = ctx.enter_context(tc.tile_pool(name="ps_tp", bufs=2, space="PSUM"))
        ps_md = ctx.enter_context(tc.tile_pool(name="ps_md", bufs=2, space="PSUM"))
        ps_o = ctx.enter_context(tc.tile_pool(name="ps_o", bufs=2, space="PSUM"))

        # ---- constants ----
        ident = sing.tile([C, C], F32)
        make_identity(nc, ident)
        ustrict = sing.tile([C, C], F32)          # u[t',t] = 1 iff t' < t
        make_upper_triangular(nc, ustrict, val=1.0, diag=False)
        uincl = sing.tile([C, C], F32)            # u[t',t] = 1 iff t' <= t
        make_upper_triangular(nc, uincl, val=1.0, diag=True)
        ones_col = sing.tile([C, 1], F32)
        nc.vector.memset(ones_col, 1.0)
        ones_row = sing.tile([1, C], F32)
        nc.vector.memset(ones_row, 1.0)
        zeros_row = sing.tile([1, NCH], F32)
        nc.vector.memset(zeros_row, 0.0)

        # steps (t+1) and their reciprocals, replicated for the 2 kinds
        steps_i = sing.tile([C, NCH], mybir.dt.int32)
        base0 = 1  # v2: per-core global offset is added via an input instead
        toff = sing.tile([C, 1], F32)  # global token offset of this core (input)
        rsteps2 = sing.tile([C, 2, NCH], F32)
        if use_cc:
            toff_in = nc.dram_tensor("toff", [1, 1], F32, kind="ExternalInput")
            nc.gpsimd.dma_start(out=toff, in_=toff_in[0:1, :].to_broadcast([C, 1]))
        else:
            nc.vector.memset(toff, 0.0)
        nc.gpsimd.iota(steps_i, pattern=[[C, NCH]], base=base0, channel_multiplier=1)
        stepsf = sing.tile([C, NCH], F32)
        nc.vector.tensor_copy(stepsf, steps_i)               # int -> float
        nc.vector.tensor_scalar_add(stepsf, stepsf, toff[:, 0:1])  # + global offset
        nc.vector.reciprocal(stepsf, stepsf)
        nc.vector.tensor_copy(rsteps2[:, 0, :], stepsf)
        nc.vector.tensor_copy(rsteps2[:, 1, :], stepsf)

        # ---- weights ----
        wproj_sb = sing.tile([C, DT, NPROJ], F32)
        nc.sync.dma_start(out=wproj_sb,
                          in_=wproj_in.rearrange("(a p) n -> p a n", p=C))
        wm_sb = sing.tile([C, WT, D], F32)
        nc.sync.dma_start(out=wm_sb, in_=wm_in.rearrange("(a p) n -> p a n", p=C))
        mix_sb = sing.tile([S, S], F32)
        nc.sync.dma_start(out=mix_sb, in_=mix_in[:, :])
        egb = sing.tile([C, S], F32)
        nc.gpsimd.dma_start(out=egb, in_=eg_in[0:1, :].to_broadcast([C, S]))
        if use_cc:
            pmask_col = sing.tile([NCORES, 1], F32)
            nc.sync.dma_start(out=pmask_col, in_=pmask_in[:, :])

        # ---- persistent per-core arrays ----
        x_all = sing.tile([C, NCH, D], F32)
        sl_all = sing.tile([C, NCH, S], F32)
        wv_all = sing.tile([C, NCH, WD], F32)
        ghdot_all = sing.tile([C, NCH], F32)
        novsal = sing.tile([C, 2, NCH], F32)
        g_all = sing.tile([C, NCH], F32)
        tg_all = sing.tile([C, NCH], F32)
        ww_all = sing.tile([C, NCH, S], F32)
        carr_i = sing.tile([1, 2, NCH], F32)
        carries = sing.tile([1, 2, NCH], F32)
        m_sb = sing.tile([S, WD], F32)
        den_sb = sing.tile([1, S], F32)

        # ================= phase 1: token-local heavy pipeline =================
        with nc.named_scope("phase1"):
            for c in range(NCH):
                xs = x_all[:, c, :]
                nc.sync.dma_start(out=xs, in_=x_in[c * C:(c + 1) * C, :])
                x0t = work.tile([C, D], F32, tag="x0")
                nc.sync.dma_start(out=x0t, in_=x0_in[c * C:(c + 1) * C, :])

                ht = work.tile([C, D], F32, tag="h")
                xsq = st.tile([C, 1], F32, tag="xsq")
                # sum(x^2) along D (ht is scratch for the squares)
                nc.scalar.activation(ht, xs, AF.Square, accum_out=xsq)
                v1t = st.tile([C, 1], F32, tag="v1")
                nc.vector.tensor_scalar(v1t, xsq, 1.0 / D, EPS,
                                        op0=ALU.mult, op1=ALU.add)
                rv = st.tile([C, 1], F32, tag="rv")
                nc.vector.reciprocal(rv, v1t)
                # salience_raw = (sum(x^2)/D) * 1/(meansq+eps)
                nc.vector.scalar_tensor_tensor(novsal[:, 1, c:c + 1], xsq, 1.0 / D,
                                               rv, op0=ALU.mult, op1=ALU.mult)
                rr = st.tile([C, 1], F32, tag="rr")
                nc.scalar.sqrt(rr, rv)                       # rsqrt(meansq+eps)
                nc.vector.tensor_scalar_mul(ht, xs, rr)      # h = x * r
                nc.gpsimd.tensor_sub(x0t, ht, x0t)           # d = h - x0
                nsum = st.tile([C, 1], F32, tag="ns")
                nc.scalar.activation(x0t, x0t, AF.Square, accum_out=nsum)
                nc.vector.tensor_scalar_mul(novsal[:, 0, c:c + 1], nsum, 1.0 / D)

                hT = work.tile([C, DT, C], F32, tag="hT")
                for dt in range(DT):
                    pt = ps_tp.tile([C, C], F32, tag="tp")
                    nc.tensor.transpose(pt, ht[:, dt * C:(dt + 1) * C], ident)
                    if dt % 2 == 0:
                        nc.vector.tensor_copy(hT[:, dt, :], pt)
                    else:
                        nc.scalar.copy(hT[:, dt, :], pt)

                pproj = ps_md.tile([C, NPROJ], F32, tag="md")
                for dt in range(DT):
                    nc.tensor.matmul(pproj, hT[:, dt, :], wproj_sb[:, dt, :],
                                     start=(dt == 0), stop=(dt == DT - 1))
                nc.scalar.copy(sl_all[:, c, :], pproj[:, 0:S])
                nc.scalar.copy(ghdot_all[:, c:c + 1], pproj[:, S:S + 1])
                nc.vector.tensor_copy(wv_all[:, c, :], pproj[:, S + 1:NPROJ])

        # ============ phase 2: running-mean chain + gates (one shot) ============
        with nc.named_scope("phase2"):
            ptot = ps_md.tile([1, 2, NCH], F32, tag="md")
            nc.tensor.matmul(ptot, ones_col, novsal, start=True, stop=True)
            if use_cc:
                # my totals = sum over chunks; AllGather; masked prefix sum
                mytot = sing.tile([1, 2], F32)
                nc.vector.tensor_reduce(mytot, ptot, axis=AX.X, op=ALU.add)
                nc.sync.dma_start(out=ag1_in[:, :], in_=mytot)
                nc.gpsimd.collective_compute(
                    "AllGather", ALU.bypass, ins=[ag1_in[:]], outs=[ag1_out[:]],
                    replica_groups=[list(range(NCORES))])
                tots8 = sing.tile([NCORES, 2], F32)
                nc.sync.dma_start(out=tots8, in_=ag1_out[:, :])
                ppref = ps_md.tile([1, 2], F32, tag="md")
                nc.tensor.matmul(ppref, pmask_col, tots8, start=True, stop=True)
                pref = sing.tile([1, 2], F32)
                nc.vector.tensor_copy(pref, ppref)
            for k in range(2):
                nc.vector.tensor_tensor_scan(carr_i[:, k, :], ptot[:, k, :],
                                             zeros_row, 0.0,
                                             op0=ALU.add, op1=ALU.add)
            nc.vector.memset(carries[:, :, 0:1], 0.0)
            for k in range(2):
                nc.vector.tensor_copy(carries[:, k, 1:NCH], carr_i[:, k, 0:NCH - 1])
            if use_cc:
                for k in range(2):
                    nc.vector.tensor_scalar_add(carries[:, k, :], carries[:, k, :],
                                                pref[:, k:k + 1])
            pcum = ps_md.tile([C, 2, NCH], F32, tag="md")
            nc.tensor.matmul(pcum, uincl, novsal, start=True, stop=False)
            nc.tensor.matmul(pcum, ones_row, carries, start=False, stop=True)
            rm = sing.tile([C, 2, NCH], F32)
            nc.vector.tensor_mul(rm, pcum, rsteps2)          # cum/(t+1)
            nc.vector.tensor_scalar_max(rm, rm, 1e-6)
            nc.vector.reciprocal(rm, rm)
            nc.vector.tensor_mul(rm, rm, novsal)             # normalized ratios
            nc.vector.tensor_add(g_all, rm[:, 0, :], rm[:, 1, :])
            g2t = sing.tile([C, NCH], F32)
            nc.vector.tensor_add(g2t, g_all, ghdot_all)
            nc.scalar.activation(tg_all, g2t, AF.Sigmoid)

        # ======= phase 2.5: write weights ww for all chunks (+ state deltas) =======
        with nc.named_scope("phase2p5"):
            for c in range(NCH):
                g_col = g_all[:, c:c + 1]
                tg_col = tg_all[:, c:c + 1]
                wl = w3.tile([C, S], F32, tag="wl")
                nc.vector.scalar_tensor_tensor(wl, egb, g_col, sl_all[:, c, :],
                                               op0=ALU.mult, op1=ALU.add)
                mx = st.tile([C, 1], F32, tag="mx")
                nc.vector.tensor_reduce(mx, wl, axis=AX.X, op=ALU.max, negate=True)
                eww = w3.tile([C, S], F32, tag="eww")
                sww = st.tile([C, 1], F32, tag="sww")
                nc.scalar.activation(eww, wl, AF.Exp, bias=mx, accum_out=sww)
                rs = st.tile([C, 1], F32, tag="rs")
                nc.vector.reciprocal(rs, sww)
                sw = st.tile([C, 1], F32, tag="sw")
                nc.vector.tensor_mul(sw, rs, tg_col)
                nc.vector.tensor_scalar_mul(ww_all[:, c, :], eww, sw)
            if use_cc:
                pmd_d = ps_md.tile([S, WD], F32, tag="md")
                pdd_d = ps_md.tile([1, S], F32, tag="md")
                for c in range(NCH):
                    nc.tensor.matmul(pmd_d, ww_all[:, c, :], wv_all[:, c, :],
                                     start=(c == 0), stop=(c == NCH - 1))
                for c in range(NCH):
                    nc.tensor.matmul(pdd_d, ones_col, ww_all[:, c, :],
                                     start=(c == 0), stop=(c == NCH - 1))
                mden_d = sing.tile([S + 1, WD], F32)
                nc.vector.tensor_copy(mden_d[0:S, :], pmd_d)
                nc.vector.memset(mden_d[S:S + 1, :], 0.0)
                nc.vector.tensor_copy(mden_d[S:S + 1, 0:S], pdd_d)
                nc.sync.dma_start(out=ag2_in[:, :], in_=mden_d)
                nc.gpsimd.collective_compute(
                    "AllGather", ALU.bypass, ins=[ag2_in[:]], outs=[ag2_out[:]],
                    replica_groups=[list(range(NCORES))])
                # gathered (k, p, d) -> sbuf [p, k, d]; M rows and den row
                # loaded into separate base-partition-0 tiles
                g2m = sing.tile([S, NCORES, WD], F32)
                nc.sync.dma_start(out=g2m,
                                  in_=ag2_out[:, 0:S, :].rearrange("k p n -> p k n"))
                g2d = sing.tile([1, NCORES, S], F32)
                nc.sync.dma_start(out=g2d, in_=ag2_out[:, S, 0:S])
                pmask_b = sing.tile([S, NCORES], F32)
                nc.gpsimd.dma_start(
                    out=pmask_b,
                    in_=pmask_in.rearrange("k one -> one k")[0:1, :]
                        .to_broadcast([S, NCORES]))
                nc.vector.memset(m_sb, 0.0)
                nc.vector.memset(den_sb, 0.0)
                tmpm = sing.tile([S, WD], F32)
                tmpd = sing.tile([1, S], F32)
                for k in range(NCORES):
                    nc.vector.tensor_scalar_mul(tmpm, g2m[:, k, :],
                                                pmask_b[:, k:k + 1])
                    nc.vector.tensor_add(m_sb, m_sb, tmpm)
                    nc.vector.tensor_scalar_mul(tmpd, g2d[:, k, :],
                                                pmask_b[0:1, k:k + 1])
                    nc.vector.tensor_add(den_sb, den_sb, tmpd)
            else:
                nc.vector.memset(m_sb, 0.0)
                nc.vector.memset(den_sb, 0.0)

        # ================= phase 3: attention + output =================
        with nc.named_scope("phase3"):
            for c in range(NCH):
                tg_col = tg_all[:, c:c + 1]
                ww = ww_all[:, c, :]
                # read weights softmax (no gate signal)
                mx2 = st.tile([C, 1], F32, tag="mx2")
                nc.vector.tensor_reduce(mx2, sl_all[:, c, :], axis=AX.X,
                                        op=ALU.max, negate=True)
                erw = w3.tile([C, S], F32, tag="erw")
                srw = st.tile([C, 1], F32, tag="srw")
                nc.scalar.activation(erw, sl_all[:, c, :], AF.Exp, bias=mx2,
                                     accum_out=srw)
                rs2 = st.tile([C, 1], F32, tag="rs2")
                nc.vector.reciprocal(rs2, srw)
                rw = w3.tile([C, S], F32, tag="rw")
                nc.vector.tensor_scalar_mul(rw, erw, rs2)
                # rw2 = rw @ mix  (token-major)
                prwT = ps_tp.tile([S, C], F32, tag="tp")
                nc.tensor.transpose(prwT, rw, ident)
                rwT = w3.tile([S, C], F32, tag="rwT")
                nc.vector.tensor_copy(rwT, prwT)
                prw2 = ps_md.tile([C, S], F32, tag="md")
                nc.tensor.matmul(prw2, rwT, mix_sb, start=True, stop=True)
                # den_ex = strict-cumsum(ww) + den_acc
                pden = ps_md.tile([C, S], F32, tag="md")
                nc.tensor.matmul(pden, ustrict, ww, start=True, stop=False)
                nc.tensor.matmul(pden, ones_row, den_sb, start=False, stop=True)
                dcl = w3.tile([C, S], F32, tag="dcl")
                nc.vector.tensor_scalar_max(dcl, pden, 1e-6)
                nc.vector.reciprocal(dcl, dcl)
                a_t = w3.tile([C, S], F32, tag="a")
                nc.vector.tensor_mul(a_t, prw2, dcl)
                # transposes for the attention matmuls
                paT = ps_tp.tile([S, C], F32, tag="tp")
                nc.tensor.transpose(paT, a_t, ident)
                aT = w3.tile([S, C], F32, tag="aT")
                nc.vector.tensor_copy(aT, paT)
                pwwT = ps_tp.tile([S, C], F32, tag="tp")
                nc.tensor.transpose(pwwT, ww, ident)
                wwT = w3.tile([S, C], F32, tag="wwT")
                nc.scalar.copy(wwT, pwwT)
                # K^T (t',t) then strict-causal mask
                pKT = ps_md.tile([C, C], F32, tag="md")
                nc.tensor.matmul(pKT, wwT, aT, start=True, stop=True)
                KTm = w3.tile([C, C], F32, tag="KTm")
                nc.vector.tensor_mul(KTm, pKT, ustrict)
                # ctx = K^T.T @ wv + a @ M
                pctx = ps_md.tile([C, WD], F32, tag="md")
                nc.tensor.matmul(pctx, KTm, wv_all[:, c, :], start=True, stop=False)
                nc.tensor.matmul(pctx, aT, m_sb, start=False, stop=True)
                ctxs = w3.tile([C, WD], F32, tag="ctx")
                nc.vector.tensor_copy(ctxs, pctx)
                # state update (after pden/pctx consumed den_sb/m_sb)
                pMd = ps_md.tile([S, WD], F32, tag="md")
                nc.tensor.matmul(pMd, ww, wv_all[:, c, :], start=True, stop=True)
                nc.vector.tensor_add(m_sb, m_sb, pMd)
                pdd = ps_md.tile([1, S], F32, tag="md")
                nc.tensor.matmul(pdd, ones_col, ww, start=True, stop=True)
                nc.vector.tensor_add(den_sb, den_sb, pdd)
                # o = ctx @ Wm'^T ;  out = x + tg*o
                ctxT = w3.tile([C, WT, C], F32, tag="ctxT")
                for wt in range(WT):
                    pcT = ps_tp.tile([C, C], F32, tag="tp")
                    nc.tensor.transpose(pcT, ctxs[:, wt * C:(wt + 1) * C], ident)
                    nc.scalar.copy(ctxT[:, wt, :], pcT)
                po = ps_o.tile([C, D], F32, tag="o")
                for nb in range(2):
                    for wt in range(WT):
                        nc.tensor.matmul(po[:, nb * 512:(nb + 1) * 512],
                                         ctxT[:, wt, :],
                                         wm_sb[:, wt, nb * 512:(nb + 1) * 512],
                                         start=(wt == 0), stop=(wt == WT - 1))
                outs = w3.tile([C, D], F32, tag="outs")
                nc.vector.scalar_tensor_tensor(outs, po, tg_col, x_all[:, c, :],
                                               op0=ALU.mult, op1=ALU.add)
                nc.sync.dma_start(out=out_dram[c * C:(c + 1) * C, :], in_=outs)

    return nc


_cache: dict = {}


def _get_nc(t_loc: int, use_cc: bool):
    key = (t_loc, use_cc)
    if key not in _cache:
        _cache[key] = _build(t_loc, use_cc)
    return _cache[key]


def kernel_with_results(x, x0, W_select, W_workspace, W_model, W_gate, slot_mix,
                        error_gain, ctm_scale, trace=False, **rkw):
    x = np.ascontiguousarray(np.asarray(x, np.float32))
    x0 = np.ascontiguousarray(np.asarray(x0, np.float32))
    wproj = np.ascontiguousarray(
        np.concatenate([np.asarray(W_select, np.float32),
                        np.asarray(W_gate, np.float32),
                        np.asarray(W_workspace, np.float32)], axis=0).T)
    wm = np.ascontiguousarray(
        (np.asarray(ctm_scale, np.float32)[:, None]
         * np.asarray(W_model, np.float32)).T)
    mix = np.ascontiguousarray(np.asarray(slot_mix, np.float32))
    eg = np.ascontiguousarray(np.asarray(error_gain, np.float32).reshape(1, S))

    t_loc = T // CORES_PER_B if USE_CC else T
    nc = _get_nc(t_loc, USE_CC)
    in_maps = []
    for k in range(NCORES):
        b, q = divmod(k, CORES_PER_B)
        sl_t = slice(q * t_loc, (q + 1) * t_loc) if USE_CC else slice(0, T)
        m = {"x": np.ascontiguousarray(x[b, sl_t]),
             "x0": np.ascontiguousarray(x0[b, sl_t]),
             "wproj": wproj, "wm": wm, "mix": mix, "eg": eg}
        if USE_CC:
            pm = np.zeros((NCORES, 1), np.float32)
            pm[b * CORES_PER_B:k, 0] = 1.0
            m["pmask"] = pm
            m["toff"] = np.full((1, 1), q * t_loc, np.float32)
        in_maps.append(m)
    res = run_bass_kernel_spmd(nc, in_maps, core_ids=list(range(NCORES)),
                               trace=trace, **rkw)
    outs = [r["out"] for r in res.results]
    if USE_CC:
        out = np.stack([np.concatenate(outs[b * CORES_PER_B:(b + 1) * CORES_PER_B],
                                       axis=0) for b in range(B)])
    else:
        out = np.stack([outs[0], outs[CORES_PER_B]])
    return out, res


def kernel(**inputs) -> np.ndarray:
    out, _ = kernel_with_results(**inputs)
    return out


# revision 17
# speedup vs baseline: 2.5578x; 1.3890x over previous
"""Trainium2 Bass kernel for nn_CTMBridge (scatter_memory).

Math: the (B,T,S,WD) cumsum "slot memory" factors into chunked linear
attention.  Per batch:
    h    = rms_norm(x)
    nov  = mean((h-x0)^2, -1);  sal = mean(h*h, -1)  (both normalized by
           running means over time -> gate signal g)
    tg   = sigmoid(h @ Wg.T + g)
    sl   = h @ Wsel.T;  ww = softmax(sl + eg*g)*tg;  rw = softmax(sl)
    wv   = h @ Wws.T
    den_ex[t,j] = sum_{t'<t} ww[t',j];   a = (rw@mix)/max(den_ex,1e-6)
    ctx[t,:] = sum_{t'<t} (a[t,:]. ww[t',:]) wv[t',:]      (linear attention)
    out  = x + ctm * tg * (ctx @ Wm.T)
Chunked over T with C=128: intra-chunk via strict-causal (C,C) matmuls,
inter-chunk via running state M (S,WD) and den (S,).

Sharding: 8 cores.  v1 (USE_CC=False): core k computes batch k//4 fully
(replicated); host picks cores 0 and 4.  v2 (USE_CC=True): each batch's
time axis is split over 4 cores; two tiny AllGathers exchange (a) the
novelty/salience running-sum prefixes and (b) the (S,WD) slot-state
prefixes between the time shards.
"""
import json
import numpy as np
from contextlib import ExitStack

import concourse.bass as bass
import concourse.mybir as mybir
import concourse.tile as tile
import concourse.bass_utils as bass_utils
import concourse.bass2jax as bass2jax
from concourse.bass_utils import run_bass_kernel_spmd
from concourse.masks import make_identity, make_upper_triangular


# --- workaround: this walrus build only accepts a single sync-wait per
# instruction on several instruction formats.  Tile attaches every needed
# cross-engine wait to the consuming instruction, so split the excess into
# standalone EventSemaphore instructions (engines are in-order, so waiting
# immediately before the instruction is equivalent).
_WAIT_LIMIT = 1


def _split_excess_waits(bir_bytes: bytes) -> bytes:
    bir = json.loads(bir_bytes)
    n = 0
    for f in bir.get("functions", []):
        for blk in f.get("blocks", []):
            out = []
            for ins in blk.get("instructions", []):
                si = ins.get("sync_info")
                waits = (si or {}).get("on_wait") or []
                eng = ins.get("engine")
                if len(waits) > _WAIT_LIMIT and eng and eng != "Unassigned":
                    for w in waits[:-_WAIT_LIMIT]:
                        n += 1
                        ev = {
                            "engine": eng,
                            "ins": [],
                            "outs": [],
                            "name": f"wsplit{n}_{ins['name']}",
                            "opcode": "EventSemaphore",
                            "sync_info": {"on_update": [], "on_wait": [w]},
                        }
                        if "debug" in ins:
                            ev["debug"] = ins["debug"]
                        out.append(ev)
                    si["on_wait"] = waits[-_WAIT_LIMIT:]
                out.append(ins)
            blk["instructions"] = out
    return json.dumps(bir).encode()


_orig_compile_bir_kernel = bass_utils.compile_bir_kernel


def _patched_compile_bir_kernel(bir_json, tmpdir, neff_name="file.neff"):
    if isinstance(bir_json, (bytes, bytearray)):
        bir_json = _split_excess_waits(bytes(bir_json))
    elif isinstance(bir_json, str):
        bir_json = _split_excess_waits(bir_json.encode()).decode()
    return _orig_compile_bir_kernel(bir_json, tmpdir, neff_name)


bass_utils.compile_bir_kernel = _patched_compile_bir_kernel
bass2jax.compile_bir_kernel = _patched_compile_bir_kernel

B, T, D, S, WD = 2, 2048, 1024, 64, 256
C = 128                      # tokens per chunk (= partition count)
DT = D // C                  # 8 d-tiles
WT = WD // C                 # 2 wd-tiles
NPROJ = S + 1 + WD           # fused projection width: [sl | ghdot | wv]
NCORES = 8
CORES_PER_B = NCORES // B
F32 = mybir.dt.float32
EPS = float(np.finfo(np.float32).eps)
AF = mybir.ActivationFunctionType
ALU = mybir.AluOpType
AX = mybir.AxisListType

USE_CC = True                # v2: time-sharded with AllGather prefix exchange


def _build(t_loc: int, use_cc: bool) -> bass.Bass:
    NCH = t_loc // C
    nc = bass.Bass()

    x_in = nc.dram_tensor("x", [t_loc, D], F32, kind="ExternalInput")
    x0_in = nc.dram_tensor("x0", [t_loc, D], F32, kind="ExternalInput")
    wproj_in = nc.dram_tensor("wproj", [D, NPROJ], F32, kind="ExternalInput")
    wm_in = nc.dram_tensor("wm", [WD, D], F32, kind="ExternalInput")
    mix_in = nc.dram_tensor("mix", [S, S], F32, kind="ExternalInput")
    eg_in = nc.dram_tensor("eg", [1, S], F32, kind="ExternalInput")
    out_dram = nc.dram_tensor("out", [t_loc, D], F32, kind="ExternalOutput")
    if use_cc:
        # pmask[k] = 1.0 for cores of the same batch whose time-slice precedes ours
        pmask_in = nc.dram_tensor("pmask", [NCORES, 1], F32, kind="ExternalInput")

    with tile.TileContext(nc) as tc, ExitStack() as ctx:
        if use_cc:
            dramp = ctx.enter_context(tc.tile_pool(name="dramp", bufs=1,
                                                   space="DRAM"))
            ag1_in = dramp.tile([1, 2], F32)
            ag1_out = dramp.tile([NCORES, 2], F32, addr_space="Shared")
            ag2_in = dramp.tile([S + 1, WD], F32)
            ag2_out = dramp.tile([NCORES, S + 1, WD], F32, addr_space="Shared")
        sing = ctx.enter_context(tc.tile_pool(name="sing", bufs=1))
        work = ctx.enter_context(tc.tile_pool(name="work", bufs=2))
        w3 = ctx.enter_context(tc.tile_pool(name="w3", bufs=2))
        st = ctx.enter_context(tc.tile_pool(name="st", bufs=3))
        ps_tp = ctx.enter_context(tc.tile_pool(name="ps_tp", bufs=2, space="PSUM"))
        ps_md = ctx.enter_context(tc.tile_pool(name="ps_md", bufs=6, space="PSUM"))

        # ---- constants ----
        ident = sing.tile([C, C], F32)
        make_identity(nc, ident)
        ustrict = sing.tile([C, C], F32)          # u[t',t] = 1 iff t' < t
        make_upper_triangular(nc, ustrict, val=1.0, diag=False)
        uincl = sing.tile([C, C], F32)            # u[t',t] = 1 iff t' <= t
        make_upper_triangular(nc, uincl, val=1.0, diag=True)
        ones_col = sing.tile([C, 1], F32)
        nc.vector.memset(ones_col, 1.0)
        ones_row = sing.tile([1, C], F32)
        nc.vector.memset(ones_row, 1.0)
        zeros_row = sing.tile([1, NCH], F32)
        nc.vector.memset(zeros_row, 0.0)
        zeros_col_s = sing.tile([S, C], F32)
        nc.vector.memset(zeros_col_s, 0.0)

        # steps (t+1) and their reciprocals, replicated for the 2 kinds
        steps_i = sing.tile([C, NCH], mybir.dt.int32)
        base0 = 1  # v2: per-core global offset is added via an input instead
        toff = sing.tile([C, 1], F32)  # global token offset of this core (input)
        rsteps2 = sing.tile([C, 2, NCH], F32)
        if use_cc:
            toff_in = nc.dram_tensor("toff", [1, 1], F32, kind="ExternalInput")
            nc.gpsimd.dma_start(out=toff, in_=toff_in[0:1, :].to_broadcast([C, 1]))
        else:
            nc.vector.memset(toff, 0.0)
        nc.gpsimd.iota(steps_i, pattern=[[C, NCH]], base=base0, channel_multiplier=1)
        stepsf = sing.tile([C, NCH], F32)
        nc.vector.tensor_copy(stepsf, steps_i)               # int -> float
        nc.vector.tensor_scalar_add(stepsf, stepsf, toff[:, 0:1])  # + global offset
        nc.vector.reciprocal(stepsf, stepsf)
        nc.vector.tensor_copy(rsteps2[:, 0, :], stepsf)
        nc.vector.tensor_copy(rsteps2[:, 1, :], stepsf)

        # ---- weights ----
        wproj_sb = sing.tile([C, DT, NPROJ], F32)
        nc.sync.dma_start(out=wproj_sb,
                          in_=wproj_in.rearrange("(a p) n -> p a n", p=C))
        wm_sb = sing.tile([C, WT, D], F32)
        nc.sync.dma_start(out=wm_sb, in_=wm_in.rearrange("(a p) n -> p a n", p=C))
        mix_sb = sing.tile([S, S], F32)
        nc.sync.dma_start(out=mix_sb, in_=mix_in[:, :])
        egb = sing.tile([C, S], F32)
        nc.gpsimd.dma_start(out=egb, in_=eg_in[0:1, :].to_broadcast([C, S]))
        if use_cc:
            pmask_col = sing.tile([NCORES, 1], F32)
            nc.sync.dma_start(out=pmask_col, in_=pmask_in[:, :])

        # ---- persistent per-core arrays ----
        x_all = sing.tile([C, NCH, D], F32)
        sl_all = sing.tile([C, NCH, S], F32)
        wv_all = sing.tile([C, NCH, WD], F32)
        ghdot_all = sing.tile([C, NCH], F32)
        novsal = sing.tile([C, 2, NCH], F32)
        g_all = sing.tile([C, NCH], F32)
        tg_all = sing.tile([C, NCH], F32)
        ww_all = sing.tile([C, NCH, S], F32)
        carr_i = sing.tile([1, 2, NCH], F32)
        carries = sing.tile([1, 2, NCH], F32)
        m_sb = sing.tile([S, WD], F32)
        den_sb = sing.tile([1, S], F32)
        den_col = sing.tile([S, 1], F32)

        # ================= phase 1: token-local heavy pipeline =================
        with nc.named_scope("phase1"):
            for c in range(NCH):
                xs = x_all[:, c, :]
                nc.sync.dma_start(out=xs, in_=x_in[c * C:(c + 1) * C, :])
                x0t = work.tile([C, D], F32, tag="x0")
                nc.sync.dma_start(out=x0t, in_=x0_in[c * C:(c + 1) * C, :])

                ht = work.tile([C, D], F32, tag="h")
                xsq = st.tile([C, 1], F32, tag="xsq")
                # sum(x^2) along D (ht is scratch for the squares)
                nc.scalar.activation(ht, xs, AF.Square, accum_out=xsq)
                v1t = st.tile([C, 1], F32, tag="v1")
                nc.vector.tensor_scalar(v1t, xsq, 1.0 / D, EPS,
                                        op0=ALU.mult, op1=ALU.add)
                rv = st.tile([C, 1], F32, tag="rv")
                nc.vector.reciprocal(rv, v1t)
                # salience_raw = (sum(x^2)/D) * 1/(meansq+eps)
                nc.vector.scalar_tensor_tensor(novsal[:, 1, c:c + 1], xsq, 1.0 / D,
                                               rv, op0=ALU.mult, op1=ALU.mult)
                rr = st.tile([C, 1], F32, tag="rr")
                nc.scalar.sqrt(rr, rv)                       # rsqrt(meansq+eps)
                nc.vector.tensor_scalar_mul(ht, xs, rr)      # h = x * r
                nc.gpsimd.tensor_sub(x0t, ht, x0t)           # d = h - x0
                nsum = st.tile([C, 1], F32, tag="ns")
                nc.scalar.activation(x0t, x0t, AF.Square, accum_out=nsum)
                nc.vector.tensor_scalar_mul(novsal[:, 0, c:c + 1], nsum, 1.0 / D)

                hT = work.tile([C, DT, C], F32, tag="hT")
                for dt in range(DT):
                    pt = ps_tp.tile([C, C], F32, tag="tp")
                    nc.tensor.transpose(pt, ht[:, dt * C:(dt + 1) * C], ident)
                    if dt % 2 == 0:
                        nc.vector.tensor_copy(hT[:, dt, :], pt)
                    else:
                        nc.scalar.copy(hT[:, dt, :], pt)

                pproj = ps_md.tile([C, NPROJ], F32, tag="md")
                for dt in range(DT):
                    nc.tensor.matmul(pproj, hT[:, dt, :], wproj_sb[:, dt, :],
                                     start=(dt == 0), stop=(dt == DT - 1))
                nc.scalar.copy(sl_all[:, c, :], pproj[:, 0:S])
                nc.scalar.copy(ghdot_all[:, c:c + 1], pproj[:, S:S + 1])
                nc.vector.tensor_copy(wv_all[:, c, :], pproj[:, S + 1:NPROJ])

        # ============ phase 2: running-mean chain + gates (one shot) ============
        with nc.named_scope("phase2"):
            ptot = ps_md.tile([1, 2, NCH], F32, tag="md")
            nc.tensor.matmul(ptot, ones_col, novsal, start=True, stop=True)
            if use_cc:
                # my totals = sum over chunks; AllGather; masked prefix sum
                mytot = sing.tile([1, 2], F32)
                nc.vector.tensor_reduce(mytot, ptot, axis=AX.X, op=ALU.add)
                nc.sync.dma_start(out=ag1_in[:, :], in_=mytot)
                nc.gpsimd.collective_compute(
                    "AllGather", ALU.bypass, ins=[ag1_in[:]], outs=[ag1_out[:]],
                    replica_groups=[list(range(NCORES))])
                tots8 = sing.tile([NCORES, 2], F32)
                nc.sync.dma_start(out=tots8, in_=ag1_out[:, :])
                ppref = ps_md.tile([1, 2], F32, tag="md")
                nc.tensor.matmul(ppref, pmask_col, tots8, start=True, stop=True)
                pref = sing.tile([1, 2], F32)
                nc.vector.tensor_copy(pref, ppref)
            for k in range(2):
                nc.vector.tensor_tensor_scan(carr_i[:, k, :], ptot[:, k, :],
                                             zeros_row, 0.0,
                                             op0=ALU.add, op1=ALU.add)
            nc.vector.memset(carries[:, :, 0:1], 0.0)
            for k in range(2):
                nc.vector.tensor_copy(carries[:, k, 1:NCH], carr_i[:, k, 0:NCH - 1])
            if use_cc:
                for k in range(2):
                    nc.vector.tensor_scalar_add(carries[:, k, :], carries[:, k, :],
                                                pref[:, k:k + 1])
            pcum = ps_md.tile([C, 2, NCH], F32, tag="md")
            nc.tensor.matmul(pcum, uincl, novsal, start=True, stop=False)
            nc.tensor.matmul(pcum, ones_row, carries, start=False, stop=True)
            rm = sing.tile([C, 2, NCH], F32)
            nc.vector.tensor_mul(rm, pcum, rsteps2)          # cum/(t+1)
            nc.vector.tensor_scalar_max(rm, rm, 1e-6)
            nc.vector.reciprocal(rm, rm)
            nc.vector.tensor_mul(rm, rm, novsal)             # normalized ratios
            nc.vector.tensor_add(g_all, rm[:, 0, :], rm[:, 1, :])
            g2t = sing.tile([C, NCH], F32)
            nc.vector.tensor_add(g2t, g_all, ghdot_all)
            nc.scalar.activation(tg_all, g2t, AF.Sigmoid)

        # ======= phase 2.5: write weights ww for all chunks (+ state deltas) =======
        with nc.named_scope("phase2p5"):
            for c in range(NCH):
                g_col = g_all[:, c:c + 1]
                tg_col = tg_all[:, c:c + 1]
                wl = w3.tile([C, S], F32, tag="wl")
                nc.vector.scalar_tensor_tensor(wl, egb, g_col, sl_all[:, c, :],
                                               op0=ALU.mult, op1=ALU.add)
                mx = st.tile([C, 1], F32, tag="mx")
                nc.vector.tensor_reduce(mx, wl, axis=AX.X, op=ALU.max, negate=True)
                eww = w3.tile([C, S], F32, tag="eww")
                sww = st.tile([C, 1], F32, tag="sww")
                nc.scalar.activation(eww, wl, AF.Exp, bias=mx, accum_out=sww)
                rs = st.tile([C, 1], F32, tag="rs")
                nc.vector.reciprocal(rs, sww)
                sw = st.tile([C, 1], F32, tag="sw")
                nc.vector.tensor_mul(sw, rs, tg_col)
                nc.vector.tensor_scalar_mul(ww_all[:, c, :], eww, sw)
            if use_cc:
                pmd_d = ps_md.tile([S, WD], F32, tag="md")
                pdd_d = ps_md.tile([1, S], F32, tag="md")
                for c in range(NCH):
                    nc.tensor.matmul(pmd_d, ww_all[:, c, :], wv_all[:, c, :],
                                     start=(c == 0), stop=(c == NCH - 1))
                for c in range(NCH):
                    nc.tensor.matmul(pdd_d, ones_col, ww_all[:, c, :],
                                     start=(c == 0), stop=(c == NCH - 1))
                mden_d = sing.tile([S + 1, WD], F32)
                nc.vector.tensor_copy(mden_d[0:S, :], pmd_d)
                nc.vector.memset(mden_d[S:S + 1, :], 0.0)
                nc.vector.tensor_copy(mden_d[S:S + 1, 0:S], pdd_d)
                nc.sync.dma_start(out=ag2_in[:, :], in_=mden_d)
                nc.gpsimd.collective_compute(
                    "AllGather", ALU.bypass, ins=[ag2_in[:]], outs=[ag2_out[:]],
                    replica_groups=[list(range(NCORES))])
                # gathered (k, p, d) -> sbuf [p, k, d]; M rows and den row
                # loaded into separate base-partition-0 tiles
                g2m = sing.tile([S, NCORES, WD], F32)
                nc.sync.dma_start(out=g2m,
                                  in_=ag2_out[:, 0:S, :].rearrange("k p n -> p k n"))
                g2d = sing.tile([1, NCORES, S], F32)
                nc.sync.dma_start(out=g2d, in_=ag2_out[:, S, 0:S])
                pmask_b = sing.tile([S, NCORES], F32)
                nc.gpsimd.dma_start(
                    out=pmask_b,
                    in_=pmask_in.rearrange("k one -> one k")[0:1, :]
                        .to_broadcast([S, NCORES]))
                nc.vector.memset(m_sb, 0.0)
                nc.vector.memset(den_sb, 0.0)
                tmpm = sing.tile([S, WD], F32)
                tmpd = sing.tile([1, S], F32)
                for k in range(NCORES):
                    nc.vector.tensor_scalar_mul(tmpm, g2m[:, k, :],
                                                pmask_b[:, k:k + 1])
                    nc.vector.tensor_add(m_sb, m_sb, tmpm)
                    nc.vector.tensor_scalar_mul(tmpd, g2d[:, k, :],
                                                pmask_b[0:1, k:k + 1])
                    nc.vector.tensor_add(den_sb, den_sb, tmpd)
            else:
                nc.vector.memset(m_sb, 0.0)
                nc.vector.memset(den_sb, 0.0)
            # den state as a (S,1) column for the free-dim scan in phase 3
            pdc = ps_tp.tile([S, 1], F32, tag="tp")
            nc.tensor.transpose(pdc, den_sb, ident[0:1, 0:1])
            nc.vector.tensor_copy(den_col, pdc)

        # ================= phase 3: attention + output =================
        # Everything slot-major (S on partitions): den_ex via a free-dim scan,
        # ctx computed directly transposed -> few PE transposes, short chains.
        with nc.named_scope("phase3"):
            for c in range(NCH):
                tg_col = tg_all[:, c:c + 1]
                ww = ww_all[:, c, :]
                # read weights softmax (no gate signal)
                mx2 = st.tile([C, 1], F32, tag="mx2")
                nc.vector.tensor_reduce(mx2, sl_all[:, c, :], axis=AX.X,
                                        op=ALU.max, negate=True)
                erw = w3.tile([C, S], F32, tag="erw")
                srw = st.tile([C, 1], F32, tag="srw")
                nc.scalar.activation(erw, sl_all[:, c, :], AF.Exp, bias=mx2,
                                     accum_out=srw)
                rs2 = st.tile([C, 1], F32, tag="rs2")
                nc.vector.reciprocal(rs2, srw)
                rw = w3.tile([C, S], F32, tag="rw")
                nc.vector.tensor_scalar_mul(rw, erw, rs2)
                # transposes: rw and ww -> slot-major
                prwT = ps_tp.tile([S, C], F32, tag="tp")
                nc.tensor.transpose(prwT, rw, ident)
                rwT = w3.tile([S, C], F32, tag="rwT")
                nc.vector.tensor_copy(rwT, prwT)
                pwwT = ps_tp.tile([S, C], F32, tag="tp")
                nc.tensor.transpose(pwwT, ww, ident)
                wwT = w3.tile([S, C], F32, tag="wwT")
                nc.scalar.copy(wwT, pwwT)
                # rw2^T = mix^T-contraction: out[j,t] = sum_i mix[i,j] rwT[i,t]
                prw2 = ps_md.tile([S, C], F32, tag="md")
                nc.tensor.matmul(prw2, mix_sb, rwT, start=True, stop=True)
                # den_ex^T via inclusive scan with initial=den_col, then shift
                scn = w3.tile([S, C], F32, tag="scn")
                nc.vector.tensor_tensor_scan(scn, wwT, zeros_col_s, den_col,
                                             op0=ALU.add, op1=ALU.add)
                dex = w3.tile([S, C], F32, tag="dex")
                nc.vector.tensor_copy(dex[:, 1:C], scn[:, 0:C - 1])
                nc.vector.tensor_copy(dex[:, 0:1], den_col)
                nc.vector.tensor_copy(den_col, scn[:, C - 1:C])  # new state
                nc.vector.tensor_scalar_max(dex, dex, 1e-6)
                nc.vector.reciprocal(dex, dex)
                aT = w3.tile([S, C], F32, tag="aT")
                nc.vector.tensor_mul(aT, prw2, dex)
                # K^T[t',t] = sum_j wwT[j,t'] aT[j,t]; strict-causal mask
                pKT = ps_md.tile([C, C], F32, tag="md")
                nc.tensor.matmul(pKT, wwT, aT, start=True, stop=True)
                KTm = w3.tile([C, C], F32, tag="KTm")
                nc.vector.tensor_mul(KTm, pKT, ustrict)
                # ctx^T directly: [w,t] = sum_t' wv[t',w] KTm[t',t] + sum_j M[j,w] aT[j,t]
                ctxT = w3.tile([C, WT, C], F32, tag="ctxT")
                for wt in range(WT):
                    pcT = ps_md.tile([C, C], F32, tag="md")
                    nc.tensor.matmul(pcT, wv_all[:, c, wt * C:(wt + 1) * C], KTm,
                                     start=True, stop=False)
                    nc.tensor.matmul(pcT, m_sb[:, wt * C:(wt + 1) * C], aT,
                                     start=False, stop=True)
                    if wt % 2 == 0:
                        nc.vector.tensor_copy(ctxT[:, wt, :], pcT)
                    else:
                        nc.scalar.copy(ctxT[:, wt, :], pcT)
                # state update (after ctx matmuls consumed m_sb)
                pMd = ps_md.tile([S, WD], F32, tag="md")
                nc.tensor.matmul(pMd, ww, wv_all[:, c, :], start=True, stop=True)
                nc.vector.tensor_add(m_sb, m_sb, pMd)
                # o = ctx @ Wm'^T ;  out = x + tg*o
                outs = w3.tile([C, D], F32, tag="outs")
                for nb in range(2):
                    po = ps_md.tile([C, 512], F32, tag="md", name=f"po{nb}")
                    for wt in range(WT):
                        nc.tensor.matmul(po,
                                         ctxT[:, wt, :],
                                         wm_sb[:, wt, nb * 512:(nb + 1) * 512],
                                         start=(wt == 0), stop=(wt == WT - 1))
                    nc.vector.scalar_tensor_tensor(
                        outs[:, nb * 512:(nb + 1) * 512], po, tg_col,
                        x_all[:, c, nb * 512:(nb + 1) * 512],
                        op0=ALU.mult, op1=ALU.add)
                nc.sync.dma_start(out=out_dram[c * C:(c + 1) * C, :], in_=outs)

    return nc


_cache: dict = {}


def _get_nc(t_loc: int, use_cc: bool):
    key = (t_loc, use_cc)
    if key not in _cache:
        _cache[key] = _build(t_loc, use_cc)
    return _cache[key]


def kernel_with_results(x, x0, W_select, W_workspace, W_model, W_gate, slot_mix,
                        error_gain, ctm_scale, trace=False, **rkw):
    x = np.ascontiguousarray(np.asarray(x, np.float32))
    x0 = np.ascontiguousarray(np.asarray(x0, np.float32))
    wproj = np.ascontiguousarray(
        np.concatenate([np.asarray(W_select, np.float32),
                        np.asarray(W_gate, np.float32),
                        np.asarray(W_workspace, np.float32)], axis=0).T)
    wm = np.ascontiguousarray(
        (np.asarray(ctm_scale, np.float32)[:, None]
         * np.asarray(W_model, np.float32)).T)
    mix = np.ascontiguousarray(np.asarray(slot_mix, np.float32))
    eg = np.ascontiguousarray(np.asarray(error_gain, np.float32).reshape(1, S))

    t_loc = T // CORES_PER_B if USE_CC else T
    nc = _get_nc(t_loc, USE_CC)
    in_maps = []
    for k in range(NCORES):
        b, q = divmod(k, CORES_PER_B)
        sl_t = slice(q * t_loc, (q + 1) * t_loc) if USE_CC else slice(0, T)
        m = {"x": np.ascontiguousarray(x[b, sl_t]),
             "x0": np.ascontiguousarray(x0[b, sl_t]),
             "wproj": wproj, "wm": wm, "mix": mix, "eg": eg}
        if USE_CC:
            pm = np.zeros((NCORES, 1), np.float32)
            pm[b * CORES_PER_B:k, 0] = 1.0
            m["pmask"] = pm
            m["toff"] = np.full((1, 1), q * t_loc, np.float32)
        in_maps.append(m)
    res = run_bass_kernel_spmd(nc, in_maps, core_ids=list(range(NCORES)),
                               trace=trace, **rkw)
    outs = [r["out"] for r in res.results]
    if USE_CC:
        out = np.stack([np.concatenate(outs[b * CORES_PER_B:(b + 1) * CORES_PER_B],
                                       axis=0) for b in range(B)])
    else:
        out = np.stack([outs[0], outs[CORES_PER_B]])
    return out, res


def kernel(**inputs) -> np.ndarray:
    out, _ = kernel_with_results(**inputs)
    return out
